# revision 86
# baseline (speedup 1.0000x reference)
"""3-layer GAT (PyG GATConv, concat=False/mean-heads) on 8 Trainium2 NeuronCores.

Strategy
--------
Destination-shard the nodes (1250 per core).  Per layer:
  1. Dense projection h_ext = x @ [W | W@a_src | W@a_dst] is computed
     REPLICATED on every core for all nodes (cheaper than all-gathering the
     23MB feature matrix through the slow ncfw collective; only the 0.6MB
     layer outputs are all-gathered between layers).  h_ext rows carry the
     per-node attention logits es/ed as 8 extra columns.
  2. Edge phase, per 128-dst block: dma_gather source rows (fp16) from the
     local h_ext copy; expand the block's per-dst logits ed to edges with a
     PE matmul against host-built transposed indicator tiles; weight each
     gathered row by ex = exp(leaky_relu(es+ed) - 3) (global shift replaces
     the segment-max; exp runs as exp((x+4)/4+bias)^4 because the ACT Exp
     LUT mishandles negative pre-bias inputs); then segment-sum onto the dst
     block with PE matmuls against 0/1 indicator tiles S[edge, dst_local];
     the denominator is the same matmul against ex.  Normalize + mean over
     heads + bias per block.
  3. AllGather the per-core output shard (transposed, fp16) to form the next
     layer's input.
Compute dtype fp16 (fp32 PSUM accumulation); ~1e-3 relative error.
"""

import numpy as np

NCORES = 8
N = 10000
E = 160000
F_IN = 512
HID = 256
H = 4
NPR = N // NCORES            # 1250 dst nodes per core
NPP = 1280                   # padded nodes per rank (10 x 128)
NP = NCORES * NPP            # 10240 padded node ids
NBLK = NPP // 128            # 10 dst blocks per core
HCOLS = H * HID              # 1024 feature cols
XCOLS = HCOLS + 2 * H        # 1032 cols with es|ed appended
GCOLS = 1152                 # h_ext row stride (fp16 rows must be 256B-granular)
SHIFT = -3.0                 # global exp shift (replaces segment max)
EPS = 1e-16
GGRP = 1024                  # edges per dma_gather call
PAD_ID = NPR                 # padded id of a guaranteed-zero row (rank 0 pad)

_cache = {}


def _pad_id(n):
    # original node id -> padded id (rank-major, 1280 slots per rank)
    return (n // NPR) * NPP + (n % NPR)


def prep_edges(edge_index):
    """Bucket edges (plus self-loops) by destination core and 128-dst block.

    Returns (EPB, per_core) where per_core[k] = (src_idx, smat, smat_t):
      src_idx: int16 [NBLK, EPB] padded gather ids
      smat:    fp16  [NBLK * NCH, 128, 128] indicator tiles S[edge, dst]
      smat_t:  fp16  same tiles transposed (S[dst, edge])
    """
    src = np.asarray(edge_index[0], dtype=np.int64)
    dst = np.asarray(edge_index[1], dtype=np.int64)
    loops = np.arange(N, dtype=np.int64)
    src = np.concatenate([src, loops])
    dst = np.concatenate([dst, loops])

    core = dst // NPR
    dloc = dst - core * NPR
    blk = dloc >> 7
    dblk = dloc & 127
    key = (core * NBLK + blk).astype(np.int64)

    order = np.argsort(key, kind="stable")
    src_s, key_s, dblk_s = src[order], key[order], dblk[order]
    counts = np.bincount(key_s, minlength=NCORES * NBLK)
    EPB = int(-(-counts.max() // 128) * 128)
    NCH = EPB // 128

    starts = np.zeros(NCORES * NBLK + 1, dtype=np.int64)
    np.cumsum(counts, out=starts[1:])

    src_pad = _pad_id(src_s).astype(np.int16)
    per_core = []
    for k in range(NCORES):
        si = np.full((NBLK, EPB), PAD_ID, dtype=np.int16)
        sm = np.zeros((NBLK * NCH, 128, 128), dtype=np.float16)
        for b in range(NBLK):
            g = k * NBLK + b
            lo, hi = starts[g], starts[g + 1]
            cnt = hi - lo
            si[b, :cnt] = src_pad[lo:hi]
            e = np.arange(cnt)
            sm[b * NCH + (e >> 7), e & 127, dblk_s[lo:hi]] = 1.0
        per_core.append((si, sm))
    return EPB, per_core


def _wrap_idx(idx, cols_per_grp, grp_elems):
    """[NB, NG*grp_elems] -> [128, NB*NG*cols] in dma_gather wrap order:
    index i of a group lives at [i % 16, grp_col_base + i // 16],
    replicated across the 8 Q7 cores (8 x 16 = 128 partitions)."""
    nb = idx.shape[0]
    ng = idx.shape[1] // grp_elems
    a = idx.reshape(nb, ng, cols_per_grp, 16)  # i = c*16 + p
    a = a.transpose(3, 0, 1, 2).reshape(16, nb * ng * cols_per_grp)
    return np.ascontiguousarray(np.tile(a, (8, 1)))


def prep_weights(W, a_s, a_d):
    """[fi, H*fo], [H, fo] x2 -> fp16 [fi//128, 128, XCOLS] with es/ed cols."""
    W = np.asarray(W, dtype=np.float64)
    fi = W.shape[0]
    fo = a_s.shape[1]
    Wh = W.reshape(fi, H, fo)
    ws = np.einsum("fhc,hc->fh", Wh, np.asarray(a_s, dtype=np.float64))
    wd = np.einsum("fhc,hc->fh", Wh, np.asarray(a_d, dtype=np.float64))
    Wx = np.concatenate([W, ws, wd], axis=1).astype(np.float16)
    return np.ascontiguousarray(Wx.reshape(fi // 128, 128, XCOLS))


def build_program(EPB, debug=False):
    import concourse.bass as bass
    import concourse.mybir as mybir
    import concourse.tile as tile
    from concourse import bacc
    from concourse.masks import make_identity

    NCH = EPB // 128
    CPG = GGRP // 128                     # chunks per full gather group
    NGRP = -(-NCH // CPG)                 # gather groups per block
    IDXW = GGRP // 16                     # idx columns per group
    fp16 = mybir.dt.float16
    f32 = mybir.dt.float32

    nc = bacc.Bacc("TRN2", target_bir_lowering=False, debug=False,
                   num_devices=NCORES, num_swdge_queues=4,
                   dynamic_dma_scratch_size=32768)

    xT0 = nc.dram_tensor("xT0", [F_IN // 128, 128, NP], fp16, kind="ExternalInput")
    Wd = [nc.dram_tensor(f"W{i}", [(F_IN if i == 0 else HID) // 128, 128, XCOLS],
                         fp16, kind="ExternalInput") for i in range(3)]
    bias_d = nc.dram_tensor("BIAS", [128, 3 * HID], f32, kind="ExternalInput")
    srcw = nc.dram_tensor("SRCIDX", [128, NBLK * NGRP * IDXW], mybir.dt.int16,
                          kind="ExternalInput")
    blkw = nc.dram_tensor("BLKIDX", [128, NBLK * 8], mybir.dt.int16,
                          kind="ExternalInput")
    smat_d = nc.dram_tensor("SMAT", [128, NBLK * NCH * 128], fp16,
                            kind="ExternalInput")
    smatt_d = nc.dram_tensor("SMATT", [128, NBLK * NCH * 128], fp16,
                             kind="ExternalInput")
    out_d = nc.dram_tensor("out", [NPR, HID], f32, kind="ExternalOutput")
    if debug:
        dbg_h = nc.dram_tensor("dbg_h", [128, XCOLS], fp16, kind="ExternalOutput")
        dbg_g = nc.dram_tensor("dbg_g", [128, 8 * GCOLS], fp16, kind="ExternalOutput")
        dbg_ex = nc.dram_tensor("dbg_ex", [128, 32], f32, kind="ExternalOutput")
        dbg_psf = nc.dram_tensor("dbg_psf", [128, HCOLS], f32, kind="ExternalOutput")
        dbg_den = nc.dram_tensor("dbg_den", [128, 4], f32, kind="ExternalOutput")
        dbg_out = nc.dram_tensor("dbg_out", [128, HID], f32, kind="ExternalOutput")

    with tile.TileContext(nc) as tc:
        with (
            tc.tile_pool(name="const", bufs=1) as constp,
            tc.tile_pool(name="lhst", bufs=8) as lhstp,
            tc.tile_pool(name="hstage", bufs=6) as hstp,
            tc.tile_pool(name="gbuf", bufs=3) as gbufp,
            tc.tile_pool(name="stt", bufs=3) as sttp,
            tc.tile_pool(name="small", bufs=8) as smallp,
            tc.tile_pool(name="gw", bufs=6) as gwp,
            tc.tile_pool(name="norm", bufs=2) as normp,
            tc.tile_pool(name="psbig", bufs=2, space="PSUM") as psbigp,
            tc.tile_pool(name="pssm", bufs=1, space="PSUM") as pssmp,
            tc.tile_pool(name="psxs", bufs=2, space="PSUM") as psxsp,
            tc.tile_pool(name="dram", bufs=1, space="DRAM") as dramp,
        ):
            # ---- resident constants ----
            w_sb = []
            for i in range(3):
                kcs = Wd[i].shape[0]
                wt = constp.tile([128, kcs, XCOLS], fp16, name=f"w{i}_sb")
                nc.sync.dma_start(wt[:], Wd[i].rearrange("kc p c -> p kc c"))
                w_sb.append(wt)
            # big resident constants go on the ACT HWDGE ring so they don't
            # block the dense phase's lhsT loads on the sync ring
            s_sb = constp.tile([128, NBLK * NCH * 128], fp16, name="s_sb")
            nc.scalar.dma_start(s_sb[:], smat_d[:])
            bias_sb = constp.tile([128, 3 * HID], f32, name="bias_sb")
            nc.scalar.dma_start(bias_sb[:], bias_d[:])
            srci = constp.tile([128, NBLK * NGRP * IDXW], mybir.dt.int16,
                               name="srci")
            nc.scalar.dma_start(srci[:], srcw[:])
            blki = constp.tile([128, NBLK * 8], mybir.dt.int16, name="blki")
            nc.scalar.dma_start(blki[:], blkw[:])
            ident = constp.tile([128, 128], fp16, name="ident")
            make_identity(nc, ident[:])
            # exp(x) is computed as exp((x+4)/4 + bias)^4 with bias=(SHIFT-4)/4:
            # the ACT Exp LUT mishandles negative pre-bias inputs, so the tile
            # fed to it carries lrelu(e)+4 >= 0 and the bias path (exact)
            # re-centers; squaring twice undoes the /4.
            shiftc = constp.tile([128, 1], f32, name="shiftc")
            nc.vector.memset(shiftc[:], (SHIFT - 4.0) / 4.0)

            h_all = dramp.tile([NP, GCOLS], fp16, name="h_all")
            HNP = NPP // 2
            ag_in = [[dramp.tile([2, 128, HNP], fp16, name=f"ag_in{i}{hf}")
                      for hf in range(2)] for i in range(2)]
            ag_out = [[dramp.tile([NCORES, 2, 128, HNP], fp16,
                                  addr_space="Shared", name=f"ag_out{i}{hf}")
                       for hf in range(2)] for i in range(2)]

            RT = NP // 128

            def dense(layer):
                """h_all[:, :XCOLS] = x @ W_ext for all nodes (replicated)."""
                kcs = F_IN // 128 if layer == 0 else HID // 128
                for rt in range(RT):
                    lt = lhstp.tile([128, kcs, 128], fp16, tag="lhst")
                    if layer == 0:
                        nc.sync.dma_start(
                            lt[:], xT0[:, :, rt * 128:(rt + 1) * 128]
                            .rearrange("kc p j -> p kc j"))
                    else:
                        rank, sub = rt // NBLK, rt % NBLK
                        hf, sc = sub // 5, sub % 5
                        nc.sync.dma_start(
                            lt[:], ag_out[layer - 1][hf]
                            [rank, :, :, sc * 128:(sc + 1) * 128]
                            .rearrange("kc p j -> p kc j"))
                    ps = psbigp.tile([128, HCOLS], f32, tag="bigp")
                    pse = psxsp.tile([128, 2 * H], f32, tag="xsp")
                    for kc in range(kcs):
                        st0 = (kc == 0)
                        sp = (kc == kcs - 1)
                        nc.tensor.matmul(ps[:, 0:512], lhsT=lt[:, kc],
                                         rhs=w_sb[layer][:, kc, 0:512],
                                         start=st0, stop=sp)
                        nc.tensor.matmul(ps[:, 512:1024], lhsT=lt[:, kc],
                                         rhs=w_sb[layer][:, kc, 512:1024],
                                         start=st0, stop=sp)
                        nc.tensor.matmul(pse[:], lhsT=lt[:, kc],
                                         rhs=w_sb[layer][:, kc, HCOLS:XCOLS],
                                         start=st0, stop=sp)
                    hs = hstp.tile([128, XCOLS], fp16, tag="hstage")
                    if rt % 2 == 0:
                        nc.vector.tensor_copy(hs[:, :HCOLS], ps[:])
                        nc.scalar.copy(hs[:, HCOLS:XCOLS], pse[:])
                    else:
                        nc.scalar.copy(hs[:, :HCOLS], ps[:])
                        nc.vector.tensor_copy(hs[:, HCOLS:XCOLS], pse[:])
                    nc.sync.dma_start(
                        h_all[rt * 128:(rt + 1) * 128, :XCOLS], hs[:])
                    if debug and layer == 0 and rt == 0:
                        nc.sync.dma_start(dbg_h[:], hs[:])

            def edge_phase(layer):
                last = layer == 2
                if not last:
                    xtn = smallp.tile([128, 2, NPP], fp16, name=f"xtn{layer}",
                                      tag="xtn", bufs=1)
                    nc.vector.memset(xtn[:], 0.0)
                for b in range(NBLK):
                    psf = psbigp.tile([128, HCOLS], f32, tag="bigp")
                    psden = pssmp.tile([128, 4], f32, tag="denp")
                    # per-dst logits for this block: one 128-row gather
                    edb = smallp.tile([128, 1, 128], fp16, tag="edb")
                    nc.gpsimd.dma_gather(
                        edb[:], h_all[:, HCOLS:GCOLS],
                        blki[:, b * 8:(b + 1) * 8],
                        num_idxs=128, num_idxs_reg=128, elem_size=128,
                        elem_step=GCOLS, queue_num=3)
                    st2 = sttp.tile([128, NCH * 128], fp16, tag="stt")
                    nc.scalar.dma_start(
                        st2[:], smatt_d[:, b * NCH * 128:(b + 1) * NCH * 128])
                    for grp in range(NGRP):
                        cpg = min(CPG, NCH - grp * CPG)
                        nidx = cpg * 128
                        icol = (b * NGRP + grp) * IDXW
                        iw = nidx // 16
                        g = gbufp.tile([128, CPG, GCOLS], fp16, tag="gbuf")
                        nc.gpsimd.dma_gather(
                            g[:, :cpg], h_all[:], srci[:, icol:icol + iw],
                            num_idxs=nidx, num_idxs_reg=nidx, elem_size=GCOLS,
                            queue_num=(b * NGRP + grp) % 3)
                        if debug and layer == 0 and b == 0 and grp == 0:
                            nc.sync.dma_start(
                                dbg_g[:], g.rearrange("p c e -> p (c e)"))
                        # expand ed to edges: psed[:, cc*4:] = S_c^T.T @ ed_blk
                        psed = psxsp.tile([128, CPG * 4], f32, tag="xsp")
                        for cc in range(cpg):
                            c = grp * CPG + cc
                            nc.tensor.matmul(
                                psed[:, cc * 4:(cc + 1) * 4],
                                lhsT=st2[:, c * 128:(c + 1) * 128],
                                rhs=edb[:, 0, 4:8], start=True, stop=True)
                        # batched per-edge scalars on contiguous [128, cpg*4]:
                        # ef = es + ed + 4 ; lr4 = max(ef, 0.2*ef + 3.2)
                        nv = cpg * 4
                        ef = smallp.tile([128, CPG * 4], f32, tag="ef")
                        nc.vector.tensor_scalar_add(
                            ef[:, :nv].rearrange("p (c e) -> p c e", e=4),
                            g[:, :cpg, HCOLS:HCOLS + 4], 4.0)
                        nc.vector.tensor_tensor(
                            ef[:, :nv], ef[:, :nv], psed[:, :nv],
                            op=mybir.AluOpType.add)
                        lr = smallp.tile([128, CPG * 4], f32, tag="lr")
                        nc.vector.tensor_scalar(
                            lr[:, :nv], ef[:, :nv], 0.2, 3.2,
                            mybir.AluOpType.mult, mybir.AluOpType.add)
                        nc.vector.tensor_tensor(
                            lr[:, :nv], lr[:, :nv], ef[:, :nv],
                            op=mybir.AluOpType.max)
                        ex = smallp.tile([128, CPG * 4], f32, tag="ex")
                        nc.scalar.activation(
                            ex[:, :nv], lr[:, :nv],
                            mybir.ActivationFunctionType.Exp,
                            bias=shiftc[:], scale=0.25)
                        nc.vector.tensor_tensor(ex[:, :nv], ex[:, :nv],
                                                ex[:, :nv],
                                                op=mybir.AluOpType.mult)
                        nc.vector.tensor_tensor(ex[:, :nv], ex[:, :nv],
                                                ex[:, :nv],
                                                op=mybir.AluOpType.mult)
                        ex16 = smallp.tile([128, CPG * 4], fp16, tag="ex16")
                        nc.vector.tensor_copy(ex16[:, :nv], ex[:, :nv])
                        if debug and layer == 0 and b == 0 and grp == 0:
                            nc.sync.dma_start(dbg_ex[:], ex[:])
                        for cc in range(cpg):
                            c = grp * CPG + cc
                            gc = g[:, cc]
                            gp = gwp.tile([128, H, HID], fp16, tag="gw")
                            for hh in range(H):
                                sca = ex[:, cc * 4 + hh:cc * 4 + hh + 1]
                                if hh % 2 == 0:
                                    nc.vector.tensor_scalar_mul(
                                        gp[:, hh], gc[:, hh * HID:(hh + 1) * HID],
                                        sca)
                                else:
                                    nc.scalar.activation(
                                        gp[:, hh], gc[:, hh * HID:(hh + 1) * HID],
                                        mybir.ActivationFunctionType.Copy,
                                        scale=sca)
                            st = s_sb[:, (b * NCH + c) * 128:(b * NCH + c + 1) * 128]
                            nc.tensor.matmul(psf[:, 0:512], lhsT=st,
                                             rhs=gp[:, 0:2],
                                             start=(c == 0), stop=(c == NCH - 1))
                            nc.tensor.matmul(psf[:, 512:1024], lhsT=st,
                                             rhs=gp[:, 2:4],
                                             start=(c == 0), stop=(c == NCH - 1))
                            nc.tensor.matmul(psden[:], lhsT=st,
                                             rhs=ex16[:, cc * 4:(cc + 1) * 4],
                                             start=(c == 0), stop=(c == NCH - 1))
                    # ---- normalize block b ----
                    if debug and layer == 0 and b == 0:
                        dpsf = normp.tile([128, HCOLS], f32, name="dpsf", bufs=1)
                        nc.vector.tensor_copy(dpsf[:], psf[:, :HCOLS])
                        nc.sync.dma_start(dbg_psf[:], dpsf[:])
                        dden = normp.tile([128, 4], f32, name="dden", bufs=1)
                        nc.vector.tensor_copy(dden[:], psden[:])
                        nc.sync.dma_start(dbg_den[:], dden[:])
                    den = normp.tile([128, 4], f32, tag="den")
                    nc.vector.tensor_scalar_add(den[:], psden[:], EPS)
                    rec = normp.tile([128, 4], f32, tag="rec")
                    nc.vector.reciprocal(rec[:], den[:])
                    nc.vector.tensor_scalar_mul(rec[:], rec[:], 1.0 / H)
                    acc = normp.tile([128, HID], f32, tag="acc")
                    t1 = normp.tile([128, HID], f32, tag="t1")
                    t2 = normp.tile([128, HID], f32, tag="t2")
                    t3 = normp.tile([128, HID], f32, tag="t3")
                    nc.vector.tensor_scalar_mul(acc[:], psf[:, 0:HID], rec[:, 0:1])
                    nc.scalar.activation(t1[:], psf[:, HID:2 * HID],
                                         mybir.ActivationFunctionType.Copy,
                                         scale=rec[:, 1:2])
                    nc.vector.tensor_scalar_mul(t2[:], psf[:, 2 * HID:3 * HID],
                                                rec[:, 2:3])
                    nc.scalar.activation(t3[:], psf[:, 3 * HID:4 * HID],
                                         mybir.ActivationFunctionType.Copy,
                                         scale=rec[:, 3:4])
                    nc.vector.tensor_add(acc[:], acc[:], t1[:])
                    nc.vector.tensor_add(t2[:], t2[:], t3[:])
                    nc.vector.tensor_add(acc[:], acc[:], t2[:])
                    bsl = bias_sb[:, layer * HID:(layer + 1) * HID]
                    if last:
                        outf = normp.tile([128, HID], f32, tag="outf")
                        nc.vector.tensor_tensor(outf[:], acc[:], bsl,
                                                op=mybir.AluOpType.add)
                        rows = NPR - b * 128 if b == NBLK - 1 else 128
                        nc.sync.dma_start(
                            out_d[b * 128:b * 128 + rows, :], outf[:rows])
                    else:
                        o16 = normp.tile([128, HID], fp16, tag="o16")
                        nc.vector.tensor_tensor(o16[:], acc[:], bsl,
                                                op=mybir.AluOpType.add)
                        if debug and layer == 0 and b == 0:
                            dout = normp.tile([128, HID], f32, name="dout", bufs=1)
                            nc.vector.tensor_tensor(dout[:], acc[:], bsl,
                                                    op=mybir.AluOpType.add)
                            nc.sync.dma_start(dbg_out[:], dout[:])
                        for ft in range(2):
                            pt = pssmp.tile([128, 128], fp16, tag="trp")
                            nc.tensor.transpose(
                                pt[:], o16[:, ft * 128:(ft + 1) * 128], ident[:])
                            nc.vector.tensor_copy(
                                xtn[:, ft, b * 128:(b + 1) * 128], pt[:])
                if not last:
                    for hf in range(2):
                        nc.sync.dma_start(
                            ag_in[layer][hf].rearrange("kc p j -> p kc j"),
                            xtn[:, :, hf * HNP:(hf + 1) * HNP])
                        nc.gpsimd.collective_compute(
                            "AllGather", mybir.AluOpType.bypass,
                            replica_groups=[list(range(NCORES))],
                            ins=[ag_in[layer][hf].opt()],
                            outs=[ag_out[layer][hf].opt()])

            for layer in range(3):
                dense(layer)
                edge_phase(layer)

    nc.compile()
    return nc


LAST_EXEC_TIME_NS = None


def kernel(x, edge_index, W0, as0, ad0, b0, W1, as1, ad1, b1, W2, as2, ad2, b2,
           *, _trace=False):
    global LAST_EXEC_TIME_NS
    from concourse.bass_utils import run_bass_kernel_spmd

    x = np.asarray(x, dtype=np.float32)
    EPB, per_core = prep_edges(edge_index)
    NCH = EPB // 128
    NGRP = -(-NCH // (GGRP // 128))

    if EPB not in _cache:
        _cache[EPB] = build_program(EPB)
    nc = _cache[EPB]

    # xT0: [F_IN//128, 128, NP] fp16, padded node cols zero
    xt = np.zeros((F_IN // 128, 128, NP), dtype=np.float16)
    xsrc = x.T.reshape(F_IN // 128, 128, NCORES, NPR)
    xt.reshape(F_IN // 128, 128, NCORES, NPP)[:, :, :, :NPR] = xsrc

    Ws = [prep_weights(W0, as0, ad0), prep_weights(W1, as1, ad1),
          prep_weights(W2, as2, ad2)]
    bias = np.zeros((128, 3 * HID), dtype=np.float32)
    for i, b in enumerate((b0, b1, b2)):
        bias[:, i * HID:(i + 1) * HID] = np.asarray(b, dtype=np.float32)[None, :]

    # pad each block's idx rows out to NGRP full groups of GGRP
    EPBG = NGRP * GGRP
    in_maps = []
    for k in range(NCORES):
        si, sm = per_core[k]
        sig = np.full((NBLK, EPBG), PAD_ID, dtype=np.int16)
        sig[:, :EPB] = si
        # per-block dst row ids (128 per block)
        bi = np.full((NBLK, 128), PAD_ID, dtype=np.int16)
        for b in range(NBLK):
            rows = min(128, NPR - b * 128)
            bi[b, :rows] = k * NPP + b * 128 + np.arange(rows, dtype=np.int16)
        smw = np.ascontiguousarray(
            sm.transpose(1, 0, 2).reshape(128, NBLK * NCH * 128))
        smwt = np.ascontiguousarray(
            sm.transpose(2, 0, 1).reshape(128, NBLK * NCH * 128))
        in_maps.append({
            "xT0": xt, "W0": Ws[0], "W1": Ws[1], "W2": Ws[2], "BIAS": bias,
            "SRCIDX": _wrap_idx(sig, GGRP // 16, GGRP),
            "BLKIDX": _wrap_idx(bi, 8, 128),
            "SMAT": smw, "SMATT": smwt,
        })

    res = run_bass_kernel_spmd(nc, in_maps, list(range(NCORES)), trace=_trace)
    LAST_EXEC_TIME_NS = res.exec_time_ns
    out = np.concatenate([res.results[k]["out"] for k in range(NCORES)], axis=0)
    return out.astype(np.float32)


# revision 87
# speedup vs baseline: 1.0043x; 1.0043x over previous
"""3-layer GAT (PyG GATConv, concat=False/mean-heads) on 8 Trainium2 NeuronCores.

Strategy
--------
Destination-shard the nodes (1250 per core).  Per layer:
  1. Dense projection h_ext = x @ [W | W@a_src | W@a_dst] is computed
     REPLICATED on every core for all nodes (cheaper than all-gathering the
     23MB feature matrix through the slow ncfw collective; only the 0.6MB
     layer outputs are all-gathered between layers).  h_ext rows carry the
     per-node attention logits es/ed as 8 extra columns.
  2. Edge phase, per 128-dst block: dma_gather source rows (fp16) from the
     local h_ext copy; expand the block's per-dst logits ed to edges with a
     PE matmul against host-built transposed indicator tiles; weight each
     gathered row by ex = exp(leaky_relu(es+ed) - 3) (global shift replaces
     the segment-max; exp runs as exp((x+4)/4+bias)^4 because the ACT Exp
     LUT mishandles negative pre-bias inputs); then segment-sum onto the dst
     block with PE matmuls against 0/1 indicator tiles S[edge, dst_local];
     the denominator is the same matmul against ex.  Normalize + mean over
     heads + bias per block.
  3. AllGather the per-core output shard (transposed, fp16) to form the next
     layer's input.
Compute dtype fp16 (fp32 PSUM accumulation); ~1e-3 relative error.
"""

import numpy as np

NCORES = 8
N = 10000
E = 160000
F_IN = 512
HID = 256
H = 4
NPR = N // NCORES            # 1250 dst nodes per core
NPP = 1280                   # padded nodes per rank (10 x 128)
NP = NCORES * NPP            # 10240 padded node ids
NBLK = NPP // 128            # 10 dst blocks per core
HCOLS = H * HID              # 1024 feature cols
XCOLS = HCOLS + 2 * H        # 1032 cols with es|ed appended
GCOLS = 1152                 # h_ext row stride (fp16 rows must be 256B-granular)
SHIFT = -3.0                 # global exp shift (replaces segment max)
EPS = 1e-16
GGRP = 1024                  # edges per dma_gather call
PAD_ID = NPR                 # padded id of a guaranteed-zero row (rank 0 pad)

_cache = {}


def _pad_id(n):
    # original node id -> padded id (rank-major, 1280 slots per rank)
    return (n // NPR) * NPP + (n % NPR)


def prep_edges(edge_index):
    """Bucket edges (plus self-loops) by destination core and 128-dst block.

    Returns (EPB, per_core) where per_core[k] = (src_idx, smat, smat_t):
      src_idx: int16 [NBLK, EPB] padded gather ids
      smat:    fp16  [NBLK * NCH, 128, 128] indicator tiles S[edge, dst]
      smat_t:  fp16  same tiles transposed (S[dst, edge])
    """
    src = np.asarray(edge_index[0], dtype=np.int64)
    dst = np.asarray(edge_index[1], dtype=np.int64)
    loops = np.arange(N, dtype=np.int64)
    src = np.concatenate([src, loops])
    dst = np.concatenate([dst, loops])

    core = dst // NPR
    dloc = dst - core * NPR
    blk = dloc >> 7
    dblk = dloc & 127
    key = (core * NBLK + blk).astype(np.int64)

    order = np.argsort(key, kind="stable")
    src_s, key_s, dblk_s = src[order], key[order], dblk[order]
    counts = np.bincount(key_s, minlength=NCORES * NBLK)
    EPB = int(-(-counts.max() // 128) * 128)
    NCH = EPB // 128

    starts = np.zeros(NCORES * NBLK + 1, dtype=np.int64)
    np.cumsum(counts, out=starts[1:])

    src_pad = _pad_id(src_s).astype(np.int16)
    per_core = []
    for k in range(NCORES):
        si = np.full((NBLK, EPB), PAD_ID, dtype=np.int16)
        sm = np.zeros((NBLK * NCH, 128, 128), dtype=np.float16)
        for b in range(NBLK):
            g = k * NBLK + b
            lo, hi = starts[g], starts[g + 1]
            cnt = hi - lo
            si[b, :cnt] = src_pad[lo:hi]
            e = np.arange(cnt)
            sm[b * NCH + (e >> 7), e & 127, dblk_s[lo:hi]] = 1.0
        per_core.append((si, sm))
    return EPB, per_core


def _wrap_idx(idx, cols_per_grp, grp_elems):
    """[NB, NG*grp_elems] -> [128, NB*NG*cols] in dma_gather wrap order:
    index i of a group lives at [i % 16, grp_col_base + i // 16],
    replicated across the 8 Q7 cores (8 x 16 = 128 partitions)."""
    nb = idx.shape[0]
    ng = idx.shape[1] // grp_elems
    a = idx.reshape(nb, ng, cols_per_grp, 16)  # i = c*16 + p
    a = a.transpose(3, 0, 1, 2).reshape(16, nb * ng * cols_per_grp)
    return np.ascontiguousarray(np.tile(a, (8, 1)))


def prep_weights(W, a_s, a_d):
    """[fi, H*fo], [H, fo] x2 -> fp16 [fi//128, 128, XCOLS] with es/ed cols."""
    W = np.asarray(W, dtype=np.float64)
    fi = W.shape[0]
    fo = a_s.shape[1]
    Wh = W.reshape(fi, H, fo)
    ws = np.einsum("fhc,hc->fh", Wh, np.asarray(a_s, dtype=np.float64))
    wd = np.einsum("fhc,hc->fh", Wh, np.asarray(a_d, dtype=np.float64))
    Wx = np.concatenate([W, ws, wd], axis=1).astype(np.float16)
    return np.ascontiguousarray(Wx.reshape(fi // 128, 128, XCOLS))


def build_program(EPB, debug=False):
    import concourse.bass as bass
    import concourse.mybir as mybir
    import concourse.tile as tile
    from concourse import bacc
    from concourse.masks import make_identity

    NCH = EPB // 128
    CPG = GGRP // 128                     # chunks per full gather group
    NGRP = -(-NCH // CPG)                 # gather groups per block
    IDXW = GGRP // 16                     # idx columns per group
    fp16 = mybir.dt.float16
    f32 = mybir.dt.float32

    nc = bacc.Bacc("TRN2", target_bir_lowering=False, debug=False,
                   num_devices=NCORES, num_swdge_queues=4,
                   dynamic_dma_scratch_size=32768)

    xT0 = nc.dram_tensor("xT0", [F_IN // 128, 128, NP], fp16, kind="ExternalInput")
    Wd = [nc.dram_tensor(f"W{i}", [(F_IN if i == 0 else HID) // 128, 128, XCOLS],
                         fp16, kind="ExternalInput") for i in range(3)]
    bias_d = nc.dram_tensor("BIAS", [128, 3 * HID], f32, kind="ExternalInput")
    srcw = nc.dram_tensor("SRCIDX", [128, NBLK * NGRP * IDXW], mybir.dt.int16,
                          kind="ExternalInput")
    blkw = nc.dram_tensor("BLKIDX", [128, NBLK * 8], mybir.dt.int16,
                          kind="ExternalInput")
    smat_d = nc.dram_tensor("SMAT", [128, NBLK * NCH * 128], fp16,
                            kind="ExternalInput")
    smatt_d = nc.dram_tensor("SMATT", [128, NBLK * NCH * 128], fp16,
                             kind="ExternalInput")
    out_d = nc.dram_tensor("out", [NPR, HID], f32, kind="ExternalOutput")
    if debug:
        dbg_h = nc.dram_tensor("dbg_h", [128, XCOLS], fp16, kind="ExternalOutput")
        dbg_g = nc.dram_tensor("dbg_g", [128, 8 * GCOLS], fp16, kind="ExternalOutput")
        dbg_ex = nc.dram_tensor("dbg_ex", [128, 32], f32, kind="ExternalOutput")
        dbg_psf = nc.dram_tensor("dbg_psf", [128, HCOLS], f32, kind="ExternalOutput")
        dbg_den = nc.dram_tensor("dbg_den", [128, 4], f32, kind="ExternalOutput")
        dbg_out = nc.dram_tensor("dbg_out", [128, HID], f32, kind="ExternalOutput")

    with tile.TileContext(nc) as tc:
        with (
            tc.tile_pool(name="const", bufs=1) as constp,
            tc.tile_pool(name="lhst", bufs=8) as lhstp,
            tc.tile_pool(name="hstage", bufs=4) as hstp,
            tc.tile_pool(name="gbuf", bufs=3) as gbufp,
            tc.tile_pool(name="stt", bufs=3) as sttp,
            tc.tile_pool(name="small", bufs=8) as smallp,
            tc.tile_pool(name="gw", bufs=6) as gwp,
            tc.tile_pool(name="norm", bufs=2) as normp,
            tc.tile_pool(name="psbig", bufs=2, space="PSUM") as psbigp,
            tc.tile_pool(name="pssm", bufs=1, space="PSUM") as pssmp,
            tc.tile_pool(name="psxs", bufs=2, space="PSUM") as psxsp,
            tc.tile_pool(name="dram", bufs=1, space="DRAM") as dramp,
        ):
            # ---- resident constants ----
            w_sb = []
            for i in range(3):
                kcs = Wd[i].shape[0]
                wt = constp.tile([128, kcs, XCOLS], fp16, name=f"w{i}_sb")
                nc.sync.dma_start(wt[:], Wd[i].rearrange("kc p c -> p kc c"))
                w_sb.append(wt)
            # big resident constants go on the ACT HWDGE ring so they don't
            # block the dense phase's lhsT loads on the sync ring
            s_sb = constp.tile([128, NBLK * NCH * 128], fp16, name="s_sb")
            nc.scalar.dma_start(s_sb[:], smat_d[:])
            bias_sb = constp.tile([128, 3 * HID], f32, name="bias_sb")
            nc.scalar.dma_start(bias_sb[:], bias_d[:])
            srci = constp.tile([128, NBLK * NGRP * IDXW], mybir.dt.int16,
                               name="srci")
            nc.scalar.dma_start(srci[:], srcw[:])
            blki = constp.tile([128, NBLK * 8], mybir.dt.int16, name="blki")
            nc.scalar.dma_start(blki[:], blkw[:])
            ident = constp.tile([128, 128], fp16, name="ident")
            make_identity(nc, ident[:])
            # exp(x) is computed as exp((x+4)/4 + bias)^4 with bias=(SHIFT-4)/4:
            # the ACT Exp LUT mishandles negative pre-bias inputs, so the tile
            # fed to it carries lrelu(e)+4 >= 0 and the bias path (exact)
            # re-centers; squaring twice undoes the /4.
            shiftc = constp.tile([128, 1], f32, name="shiftc")
            nc.vector.memset(shiftc[:], (SHIFT - 4.0) / 4.0)

            h_all = dramp.tile([NP, GCOLS], fp16, name="h_all")
            HNP = NPP // 2
            ag_in = [[dramp.tile([2, 128, HNP], fp16, name=f"ag_in{i}{hf}")
                      for hf in range(2)] for i in range(2)]
            ag_out = [[dramp.tile([NCORES, 2, 128, HNP], fp16,
                                  addr_space="Shared", name=f"ag_out{i}{hf}")
                       for hf in range(2)] for i in range(2)]

            RT = NP // 128

            def dense(layer):
                """h_all[:, :XCOLS] = x @ W_ext for all nodes (replicated)."""
                kcs = F_IN // 128 if layer == 0 else HID // 128
                for rt in range(RT):
                    lt = lhstp.tile([128, kcs, 128], fp16, tag="lhst")
                    if layer == 0:
                        nc.sync.dma_start(
                            lt[:], xT0[:, :, rt * 128:(rt + 1) * 128]
                            .rearrange("kc p j -> p kc j"))
                    else:
                        rank, sub = rt // NBLK, rt % NBLK
                        hf, sc = sub // 5, sub % 5
                        nc.sync.dma_start(
                            lt[:], ag_out[layer - 1][hf]
                            [rank, :, :, sc * 128:(sc + 1) * 128]
                            .rearrange("kc p j -> p kc j"))
                    ps = psbigp.tile([128, HCOLS], f32, tag="bigp")
                    pse = psxsp.tile([128, 2 * H], f32, tag="xsp")
                    for kc in range(kcs):
                        st0 = (kc == 0)
                        sp = (kc == kcs - 1)
                        nc.tensor.matmul(ps[:, 0:512], lhsT=lt[:, kc],
                                         rhs=w_sb[layer][:, kc, 0:512],
                                         start=st0, stop=sp)
                        nc.tensor.matmul(ps[:, 512:1024], lhsT=lt[:, kc],
                                         rhs=w_sb[layer][:, kc, 512:1024],
                                         start=st0, stop=sp)
                        nc.tensor.matmul(pse[:], lhsT=lt[:, kc],
                                         rhs=w_sb[layer][:, kc, HCOLS:XCOLS],
                                         start=st0, stop=sp)
                    hs = hstp.tile([128, XCOLS], fp16, tag="hstage")
                    if rt % 2 == 0:
                        nc.vector.tensor_copy(hs[:, :HCOLS], ps[:])
                        nc.scalar.copy(hs[:, HCOLS:XCOLS], pse[:])
                    else:
                        nc.scalar.copy(hs[:, :HCOLS], ps[:])
                        nc.vector.tensor_copy(hs[:, HCOLS:XCOLS], pse[:])
                    nc.sync.dma_start(
                        h_all[rt * 128:(rt + 1) * 128, :XCOLS], hs[:])
                    if debug and layer == 0 and rt == 0:
                        nc.sync.dma_start(dbg_h[:], hs[:])

            def edge_phase(layer):
                last = layer == 2
                if not last:
                    xtn = smallp.tile([128, 2, NPP], fp16, name=f"xtn{layer}",
                                      tag="xtn", bufs=1)
                    nc.vector.memset(xtn[:], 0.0)
                for b in range(NBLK):
                    psf = psbigp.tile([128, HCOLS], f32, tag="bigp")
                    psden = pssmp.tile([128, 4], f32, tag="denp")
                    # per-dst logits for this block: one 128-row gather
                    edb = smallp.tile([128, 1, 128], fp16, tag="edb")
                    nc.gpsimd.dma_gather(
                        edb[:], h_all[:, HCOLS:GCOLS],
                        blki[:, b * 8:(b + 1) * 8],
                        num_idxs=128, num_idxs_reg=128, elem_size=128,
                        elem_step=GCOLS, queue_num=3)
                    st2 = sttp.tile([128, NCH * 128], fp16, tag="stt")
                    nc.scalar.dma_start(
                        st2[:], smatt_d[:, b * NCH * 128:(b + 1) * NCH * 128])
                    for grp in range(NGRP):
                        cpg = min(CPG, NCH - grp * CPG)
                        nidx = cpg * 128
                        icol = (b * NGRP + grp) * IDXW
                        iw = nidx // 16
                        g = gbufp.tile([128, CPG, GCOLS], fp16, tag="gbuf")
                        nc.gpsimd.dma_gather(
                            g[:, :cpg], h_all[:], srci[:, icol:icol + iw],
                            num_idxs=nidx, num_idxs_reg=nidx, elem_size=GCOLS,
                            queue_num=(b * NGRP + grp) % 3)
                        if debug and layer == 0 and b == 0 and grp == 0:
                            nc.sync.dma_start(
                                dbg_g[:], g.rearrange("p c e -> p (c e)"))
                        # expand ed to edges: psed[:, cc*4:] = S_c^T.T @ ed_blk
                        psed = psxsp.tile([128, CPG * 4], f32, tag="xsp")
                        for cc in range(cpg):
                            c = grp * CPG + cc
                            nc.tensor.matmul(
                                psed[:, cc * 4:(cc + 1) * 4],
                                lhsT=st2[:, c * 128:(c + 1) * 128],
                                rhs=edb[:, 0, 4:8], start=True, stop=True)
                        # batched per-edge scalars on contiguous [128, cpg*4]:
                        # ef = es + ed + 4 ; lr4 = max(ef, 0.2*ef + 3.2)
                        nv = cpg * 4
                        ef = smallp.tile([128, CPG * 4], f32, tag="ef")
                        nc.vector.tensor_scalar_add(
                            ef[:, :nv].rearrange("p (c e) -> p c e", e=4),
                            g[:, :cpg, HCOLS:HCOLS + 4], 4.0)
                        nc.vector.tensor_tensor(
                            ef[:, :nv], ef[:, :nv], psed[:, :nv],
                            op=mybir.AluOpType.add)
                        lr = smallp.tile([128, CPG * 4], f32, tag="lr")
                        nc.vector.tensor_scalar(
                            lr[:, :nv], ef[:, :nv], 0.2, 3.2,
                            mybir.AluOpType.mult, mybir.AluOpType.add)
                        nc.vector.tensor_tensor(
                            lr[:, :nv], lr[:, :nv], ef[:, :nv],
                            op=mybir.AluOpType.max)
                        ex = smallp.tile([128, CPG * 4], f32, tag="ex")
                        nc.scalar.activation(
                            ex[:, :nv], lr[:, :nv],
                            mybir.ActivationFunctionType.Exp,
                            bias=shiftc[:], scale=0.25)
                        nc.vector.tensor_tensor(ex[:, :nv], ex[:, :nv],
                                                ex[:, :nv],
                                                op=mybir.AluOpType.mult)
                        nc.vector.tensor_tensor(ex[:, :nv], ex[:, :nv],
                                                ex[:, :nv],
                                                op=mybir.AluOpType.mult)
                        ex16 = smallp.tile([128, CPG * 4], fp16, tag="ex16")
                        nc.vector.tensor_copy(ex16[:, :nv], ex[:, :nv])
                        if debug and layer == 0 and b == 0 and grp == 0:
                            nc.sync.dma_start(dbg_ex[:], ex[:])
                        for cc in range(cpg):
                            c = grp * CPG + cc
                            gc = g[:, cc]
                            gp = gwp.tile([128, H, HID], fp16, tag="gw")
                            for hh in range(H):
                                sca = ex[:, cc * 4 + hh:cc * 4 + hh + 1]
                                if hh % 2 == 0:
                                    nc.vector.tensor_scalar_mul(
                                        gp[:, hh], gc[:, hh * HID:(hh + 1) * HID],
                                        sca)
                                else:
                                    nc.scalar.activation(
                                        gp[:, hh], gc[:, hh * HID:(hh + 1) * HID],
                                        mybir.ActivationFunctionType.Copy,
                                        scale=sca)
                            st = s_sb[:, (b * NCH + c) * 128:(b * NCH + c + 1) * 128]
                            nc.tensor.matmul(psf[:, 0:512], lhsT=st,
                                             rhs=gp[:, 0:2],
                                             start=(c == 0), stop=(c == NCH - 1))
                            nc.tensor.matmul(psf[:, 512:1024], lhsT=st,
                                             rhs=gp[:, 2:4],
                                             start=(c == 0), stop=(c == NCH - 1))
                            nc.tensor.matmul(psden[:], lhsT=st,
                                             rhs=ex16[:, cc * 4:(cc + 1) * 4],
                                             start=(c == 0), stop=(c == NCH - 1))
                    # ---- normalize block b ----
                    if debug and layer == 0 and b == 0:
                        dpsf = normp.tile([128, HCOLS], f32, name="dpsf", bufs=1)
                        nc.vector.tensor_copy(dpsf[:], psf[:, :HCOLS])
                        nc.sync.dma_start(dbg_psf[:], dpsf[:])
                        dden = normp.tile([128, 4], f32, name="dden", bufs=1)
                        nc.vector.tensor_copy(dden[:], psden[:])
                        nc.sync.dma_start(dbg_den[:], dden[:])
                    den = normp.tile([128, 4], f32, tag="den")
                    nc.vector.tensor_scalar_add(den[:], psden[:], EPS)
                    rec = normp.tile([128, 4], f32, tag="rec")
                    nc.vector.reciprocal(rec[:], den[:])
                    nc.vector.tensor_scalar_mul(rec[:], rec[:], 1.0 / H)
                    acc = normp.tile([128, HID], f32, tag="acc")
                    t1 = normp.tile([128, HID], f32, tag="t1")
                    t2 = normp.tile([128, HID], f32, tag="t2")
                    t3 = normp.tile([128, HID], f32, tag="t3")
                    nc.vector.tensor_scalar_mul(acc[:], psf[:, 0:HID], rec[:, 0:1])
                    nc.scalar.activation(t1[:], psf[:, HID:2 * HID],
                                         mybir.ActivationFunctionType.Copy,
                                         scale=rec[:, 1:2])
                    nc.vector.tensor_scalar_mul(t2[:], psf[:, 2 * HID:3 * HID],
                                                rec[:, 2:3])
                    nc.scalar.activation(t3[:], psf[:, 3 * HID:4 * HID],
                                         mybir.ActivationFunctionType.Copy,
                                         scale=rec[:, 3:4])
                    nc.vector.tensor_add(acc[:], acc[:], t1[:])
                    nc.vector.tensor_add(t2[:], t2[:], t3[:])
                    nc.vector.tensor_add(acc[:], acc[:], t2[:])
                    bsl = bias_sb[:, layer * HID:(layer + 1) * HID]
                    if last:
                        outf = normp.tile([128, HID], f32, tag="outf")
                        nc.vector.tensor_tensor(outf[:], acc[:], bsl,
                                                op=mybir.AluOpType.add)
                        rows = NPR - b * 128 if b == NBLK - 1 else 128
                        nc.sync.dma_start(
                            out_d[b * 128:b * 128 + rows, :], outf[:rows])
                    else:
                        o16 = normp.tile([128, HID], fp16, tag="o16")
                        nc.vector.tensor_tensor(o16[:], acc[:], bsl,
                                                op=mybir.AluOpType.add)
                        if debug and layer == 0 and b == 0:
                            dout = normp.tile([128, HID], f32, name="dout", bufs=1)
                            nc.vector.tensor_tensor(dout[:], acc[:], bsl,
                                                    op=mybir.AluOpType.add)
                            nc.sync.dma_start(dbg_out[:], dout[:])
                        for ft in range(2):
                            pt = pssmp.tile([128, 128], fp16, tag="trp")
                            nc.tensor.transpose(
                                pt[:], o16[:, ft * 128:(ft + 1) * 128], ident[:])
                            nc.vector.tensor_copy(
                                xtn[:, ft, b * 128:(b + 1) * 128], pt[:])
                if not last:
                    for hf in range(2):
                        nc.sync.dma_start(
                            ag_in[layer][hf].rearrange("kc p j -> p kc j"),
                            xtn[:, :, hf * HNP:(hf + 1) * HNP])
                        nc.gpsimd.collective_compute(
                            "AllGather", mybir.AluOpType.bypass,
                            replica_groups=[list(range(NCORES))],
                            ins=[ag_in[layer][hf].opt()],
                            outs=[ag_out[layer][hf].opt()])

            for layer in range(3):
                dense(layer)
                edge_phase(layer)

    nc.compile()
    return nc


LAST_EXEC_TIME_NS = None


def kernel(x, edge_index, W0, as0, ad0, b0, W1, as1, ad1, b1, W2, as2, ad2, b2,
           *, _trace=False):
    global LAST_EXEC_TIME_NS
    from concourse.bass_utils import run_bass_kernel_spmd

    x = np.asarray(x, dtype=np.float32)
    EPB, per_core = prep_edges(edge_index)
    NCH = EPB // 128
    NGRP = -(-NCH // (GGRP // 128))

    if EPB not in _cache:
        _cache[EPB] = build_program(EPB)
    nc = _cache[EPB]

    # xT0: [F_IN//128, 128, NP] fp16, padded node cols zero
    xt = np.zeros((F_IN // 128, 128, NP), dtype=np.float16)
    xsrc = x.T.reshape(F_IN // 128, 128, NCORES, NPR)
    xt.reshape(F_IN // 128, 128, NCORES, NPP)[:, :, :, :NPR] = xsrc

    Ws = [prep_weights(W0, as0, ad0), prep_weights(W1, as1, ad1),
          prep_weights(W2, as2, ad2)]
    bias = np.zeros((128, 3 * HID), dtype=np.float32)
    for i, b in enumerate((b0, b1, b2)):
        bias[:, i * HID:(i + 1) * HID] = np.asarray(b, dtype=np.float32)[None, :]

    # pad each block's idx rows out to NGRP full groups of GGRP
    EPBG = NGRP * GGRP
    in_maps = []
    for k in range(NCORES):
        si, sm = per_core[k]
        sig = np.full((NBLK, EPBG), PAD_ID, dtype=np.int16)
        sig[:, :EPB] = si
        # per-block dst row ids (128 per block)
        bi = np.full((NBLK, 128), PAD_ID, dtype=np.int16)
        for b in range(NBLK):
            rows = min(128, NPR - b * 128)
            bi[b, :rows] = k * NPP + b * 128 + np.arange(rows, dtype=np.int16)
        smw = np.ascontiguousarray(
            sm.transpose(1, 0, 2).reshape(128, NBLK * NCH * 128))
        smwt = np.ascontiguousarray(
            sm.transpose(2, 0, 1).reshape(128, NBLK * NCH * 128))
        in_maps.append({
            "xT0": xt, "W0": Ws[0], "W1": Ws[1], "W2": Ws[2], "BIAS": bias,
            "SRCIDX": _wrap_idx(sig, GGRP // 16, GGRP),
            "BLKIDX": _wrap_idx(bi, 8, 128),
            "SMAT": smw, "SMATT": smwt,
        })

    res = run_bass_kernel_spmd(nc, in_maps, list(range(NCORES)), trace=_trace)
    LAST_EXEC_TIME_NS = res.exec_time_ns
    out = np.concatenate([res.results[k]["out"] for k in range(NCORES)], axis=0)
    return out.astype(np.float32)
